# revision 6
# baseline (speedup 1.0000x reference)
"""PartTokenizer Trainium2 kernel.

Data-parallel over batch: 16 samples -> 8 cores x 2 samples (48 parts/core).

Math notes (vs the reference):
  - softmax over a size-1 axis is exactly 1.0 -> aggW/aggB drop out.
  - enc L3 is linear (no relu), so mean-over-points commutes with it:
      visual = flip * (mean_pts(relu(L2)) @ encW3 + encB3)
    The per-part point-sums ride along as `accum_out` of the L2 eviction op.
  - biases of the final projection enter as a ones-row matmul into PSUM.

Layout: encoder runs feature-major ([feat, points] on chip). The host
pre-transposes points into 4 partition row-groups (base partitions
0/32/64/96) so L1 (K=3) matmuls can use PE row-group tiling.
"""

import numpy as np

import concourse.bass as bass
import concourse.tile as tile
from concourse import bacc, mybir
from concourse.masks import make_identity

F32 = mybir.dt.float32
F32R = mybir.dt.float32r
ALU = mybir.AluOpType
ACTF = mybir.ActivationFunctionType

NCORES = 8
B, P, NS, CC = 16, 24, 512, 3
BPC = B // NCORES          # samples per core
M = BPC * P                # 48 parts per core
NG = 4                     # PE row groups used by L1
PPG = M // NG              # 12 parts per row group


def build_program():
    # float32r matmul operands: 1 cyc/row at N>=256 (vs 4 for f32). Walrus
    # requires f32r matmul inputs to be *produced* as f32r, so every tensor
    # feeding a matmul is declared f32r end to end (same 4-byte layout).
    nc = bacc.Bacc(None, target_bir_lowering=False)

    xt = nc.dram_tensor("xt", [NG, 3, PPG * NS], F32R, kind="ExternalInput")
    xbb = nc.dram_tensor("xbb", [12, M], F32R, kind="ExternalInput")
    w1blk = nc.dram_tensor("w1blk", [128, 128], F32R, kind="ExternalInput")
    w2 = nc.dram_tensor("w2", [128, 128], F32R, kind="ExternalInput")
    w3 = nc.dram_tensor("w3", [128, 128], F32R, kind="ExternalInput")
    pwbb = nc.dram_tensor("pwbb", [32, 512], F32R, kind="ExternalInput")
    pwvis = nc.dram_tensor("pwvis", [128, 512], F32R, kind="ExternalInput")
    bbw1 = nc.dram_tensor("bbw1", [12, 64], F32R, kind="ExternalInput")
    bbw2 = nc.dram_tensor("bbw2", [64, 64], F32R, kind="ExternalInput")
    bbw3 = nc.dram_tensor("bbw3", [64, 32], F32R, kind="ExternalInput")
    bvec = nc.dram_tensor("bvec", [128, 8], F32, kind="ExternalInput")
    pb = nc.dram_tensor("pb", [1, 512], F32R, kind="ExternalInput")
    flip = nc.dram_tensor("flip", [128, M], F32, kind="ExternalInput")

    o_out = nc.dram_tensor("o_out", [M, 512], F32, kind="ExternalOutput")
    o_vt = nc.dram_tensor("o_vt", [M, 128], F32, kind="ExternalOutput")
    o_bt = nc.dram_tensor("o_bt", [M, 32], F32, kind="ExternalOutput")

    with tile.TileContext(nc) as tc:
        with (
            tc.tile_pool(name="const", bufs=1) as const,
            tc.tile_pool(name="f1p", bufs=3) as f1p,
            tc.tile_pool(name="f2p", bufs=2) as f2p,
            tc.tile_pool(name="ps1", bufs=2, space="PSUM") as ps1,
            tc.tile_pool(name="ps2", bufs=2, space="PSUM") as ps2,
            tc.tile_pool(name="pst", bufs=2, space="PSUM") as pst,
            tc.tile_pool(name="pso", bufs=1, space="PSUM") as pso,
            tc.tile_pool(name="outs", bufs=1) as outs,
        ):
            # ---- constants / inputs to SBUF ----
            xbb_sb = const.tile([12, M], F32R)
            nc.sync.dma_start(out=xbb_sb, in_=xbb[:, :])
            bbw1_sb = const.tile([12, 64], F32R)
            nc.sync.dma_start(out=bbw1_sb, in_=bbw1[:, :])
            bbw2_sb = const.tile([64, 64], F32R)
            nc.sync.dma_start(out=bbw2_sb, in_=bbw2[:, :])
            bbw3_sb = const.tile([64, 32], F32R)
            nc.sync.dma_start(out=bbw3_sb, in_=bbw3[:, :])
            bvec_sb = const.tile([128, 8], F32)
            nc.sync.dma_start(out=bvec_sb, in_=bvec[:, :])
            flip_sb = const.tile([128, M], F32)
            nc.sync.dma_start(out=flip_sb, in_=flip[:, :])
            w1_sb = const.tile([128, 128], F32R)
            nc.sync.dma_start(out=w1_sb, in_=w1blk[:, :])
            w2_sb = const.tile([128, 128], F32R)
            nc.sync.dma_start(out=w2_sb, in_=w2[:, :])
            w3_sb = const.tile([128, 128], F32R)
            nc.sync.dma_start(out=w3_sb, in_=w3[:, :])
            pwbb_sb = const.tile([32, 512], F32R)
            nc.sync.dma_start(out=pwbb_sb, in_=pwbb[:, :])
            pwvis_sb = const.tile([128, 512], F32R)
            nc.sync.dma_start(out=pwvis_sb, in_=pwvis[:, :])
            pb_sb = const.tile([1, 512], F32R)
            nc.sync.dma_start(out=pb_sb, in_=pb[:, :])
            xt_sb = const.tile([128, PPG * NS], F32R)
            for r in range(NG):
                nc.sync.dma_start(out=xt_sb[32 * r : 32 * r + 3, :], in_=xt[r])

            ones_f32 = const.tile([1, M], F32)
            nc.vector.memset(ones_f32, 1.0)
            ones_sb = const.tile([1, M], F32R)
            nc.vector.tensor_copy(ones_sb, ones_f32)
            zeros_sb = const.tile([128, NS], F32)
            nc.vector.memset(zeros_sb, 0.0)
            ident_f32 = const.tile([128, 128], F32)
            make_identity(nc, ident_f32)
            ident_sb = const.tile([128, 128], F32R)
            nc.vector.tensor_copy(ident_sb, ident_f32)

            # ---- bb tokenizer chain (small; also warms the PE early) ----
            pbb1 = pst.tile([64, M], F32, tag="pst")
            nc.tensor.matmul(pbb1, bbw1_sb, xbb_sb)
            h1bb = const.tile([64, M], F32R)
            nc.scalar.activation(h1bb, pbb1, ACTF.Relu, bias=bvec_sb[0:64, 3:4])
            pbb2 = pst.tile([64, M], F32, tag="pst")
            nc.tensor.matmul(pbb2, bbw2_sb, h1bb)
            h2bb = const.tile([64, M], F32R)
            nc.scalar.activation(h2bb, pbb2, ACTF.Relu, bias=bvec_sb[0:64, 4:5])
            pbb3 = pst.tile([32, M], F32, tag="pst")
            nc.tensor.matmul(pbb3, bbw3_sb, h2bb)
            bb_fm = const.tile([32, M], F32R)
            nc.scalar.activation(bb_fm, pbb3, ACTF.Identity, bias=bvec_sb[0:32, 5:6])
            nc.vector.tensor_mul(bb_fm, bb_fm, flip_sb[0:32, :])

            # ---- encoder main loop: one part (512 points) at a time ----
            # Eviction engine balance: ACT ~570ns/tile, DVE ~658ns/tile.
            # ACT takes all L1 evictions plus a few L2s; DVE takes the rest.
            act_l2 = {15, 31, 47}
            g_sb = const.tile([128, M], F32)  # per-part sums of relu(L2)
            for m in range(M):
                r, j = divmod(m, PPG)
                base = 32 * r
                p1 = ps1.tile([128, NS], F32, tag="p1")
                nc.tensor.matmul(
                    p1,
                    w1_sb[base : base + 3, :],
                    xt_sb[base : base + 3, j * NS : (j + 1) * NS],
                    tile_position=(base, 0),
                )
                f1 = f1p.tile([128, NS], F32R, tag="f1")
                nc.scalar.activation(f1, p1, ACTF.Relu, bias=bvec_sb[:, 0:1])
                p2 = ps2.tile([128, NS], F32, tag="p2")
                nc.tensor.matmul(p2, w2_sb, f1)
                f2 = f2p.tile([128, NS], F32, tag="f2")
                if m in act_l2:
                    nc.scalar.activation(
                        f2,
                        p2,
                        ACTF.Relu,
                        bias=bvec_sb[:, 1:2],
                        accum_out=g_sb[:, m : m + 1],
                    )
                else:
                    nc.vector.scalar_tensor_tensor(
                        out=f2,
                        in0=p2,
                        scalar=bvec_sb[:, 1:2],
                        in1=zeros_sb,
                        op0=ALU.add,
                        op1=ALU.max,
                        accum_out=g_sb[:, m : m + 1],
                    )

            # ---- visual tokens: (sums/512) @ encW3 + encB3, masked ----
            g_r = const.tile([128, M], F32R)
            nc.vector.tensor_copy(g_r, g_sb)
            pv = pst.tile([128, M], F32, tag="pst")
            nc.tensor.matmul(pv, w3_sb, g_r)
            vis_fm = const.tile([128, M], F32R)
            nc.scalar.activation(
                vis_fm, pv, ACTF.Identity, bias=bvec_sb[:, 2:3], scale=1.0 / NS
            )
            nc.vector.tensor_mul(vis_fm, vis_fm, flip_sb)

            # ---- projection: out = [bb|vis] @ projW + projB ----
            po = pso.tile([M, 512], F32, tag="po")
            nc.tensor.matmul(po, bb_fm, pwbb_sb, start=True, stop=False)
            nc.tensor.matmul(po, vis_fm, pwvis_sb, start=False, stop=False)
            nc.tensor.matmul(po, ones_sb, pb_sb, start=False, stop=True)
            out_sb = outs.tile([M, 512], F32)
            nc.scalar.copy(out_sb, po)
            nc.sync.dma_start(out=o_out[:, :], in_=out_sb)

            # ---- emit tokens (transpose back to parts-major) ----
            pt1 = pst.tile([M, 128], F32R, tag="pst")
            nc.tensor.transpose(pt1, vis_fm, ident_sb)
            vt_sb = outs.tile([M, 128], F32)
            nc.vector.tensor_copy(vt_sb, pt1)
            nc.sync.dma_start(out=o_vt[:, :], in_=vt_sb)

            pt2 = pst.tile([M, 32], F32R, tag="pst")
            nc.tensor.transpose(pt2, bb_fm, ident_sb[0:32, 0:32])
            bt_sb = outs.tile([M, 32], F32)
            nc.vector.tensor_copy(bt_sb, pt2)
            nc.sync.dma_start(out=o_bt[:, :], in_=bt_sb)

    nc.compile()
    return nc


def make_in_maps(inputs):
    """Shard + lay out the full inputs for the 8 cores."""
    pts = np.ascontiguousarray(inputs["part_points"][:, :, :NS, :], np.float32)
    bbs = np.asarray(inputs["part_bbs"], np.float32)
    flipped = (~np.asarray(inputs["batch_mask"])).astype(np.float32)

    w1b = np.zeros((128, 128), np.float32)
    for r in range(NG):
        w1b[32 * r : 32 * r + 3, :] = inputs["encW1"]
    bvec = np.zeros((128, 8), np.float32)
    bvec[:, 0] = inputs["encB1"]
    bvec[:, 1] = inputs["encB2"]
    bvec[:, 2] = inputs["encB3"]
    bvec[0:64, 3] = inputs["bbB1"]
    bvec[0:64, 4] = inputs["bbB2"]
    bvec[0:32, 5] = inputs["bbB3"]
    pw = np.asarray(inputs["projW"], np.float32)
    shared = {
        "w1blk": w1b,
        "w2": np.ascontiguousarray(inputs["encW2"], np.float32),
        "w3": np.ascontiguousarray(inputs["encW3"], np.float32),
        "pwbb": np.ascontiguousarray(pw[0:32]),
        "pwvis": np.ascontiguousarray(pw[32:160]),
        "bbw1": np.ascontiguousarray(inputs["bbW1"], np.float32),
        "bbw2": np.ascontiguousarray(inputs["bbW2"], np.float32),
        "bbw3": np.ascontiguousarray(inputs["bbW3"], np.float32),
        "bvec": bvec,
        "pb": np.ascontiguousarray(inputs["projB"], np.float32).reshape(1, 512),
    }

    in_maps = []
    for c in range(NCORES):
        lo = c * BPC
        p_c = pts[lo : lo + BPC].reshape(M, NS, CC).transpose(0, 2, 1)  # [48,3,512]
        xt = np.ascontiguousarray(
            p_c.reshape(NG, PPG, 3, NS).transpose(0, 2, 1, 3).reshape(NG, 3, PPG * NS)
        )
        xbb = np.ascontiguousarray(bbs[lo : lo + BPC].reshape(M, 12).T)
        flip = np.ascontiguousarray(
            np.broadcast_to(flipped[lo : lo + BPC].reshape(1, M), (128, M))
        )
        in_maps.append({"xt": xt, "xbb": xbb, "flip": flip, **shared})
    return in_maps


_CACHE = {}


def _get_runner():
    """Build + compile once, return a persistent callable in_maps -> results."""
    if "runner" in _CACHE:
        return _CACHE["runner"]

    import jax
    from jax.sharding import Mesh, PartitionSpec
    from jax.experimental.shard_map import shard_map
    from concourse import bass2jax

    nc = build_program()
    bass2jax.install_neuronx_cc_hook()

    partition_name = nc.partition_id_tensor.name if nc.partition_id_tensor else None
    in_names, out_names, out_avals = [], [], []
    for alloc in nc.m.functions[0].allocations:
        if not isinstance(alloc, mybir.MemoryLocationSet):
            continue
        name = alloc.memorylocations[0].name
        if alloc.kind == "ExternalInput":
            if name != partition_name:
                in_names.append(name)
        elif alloc.kind == "ExternalOutput":
            out_names.append(name)
            out_avals.append(
                jax.core.ShapedArray(tuple(alloc.tensor_shape), mybir.dt.np(alloc.dtype))
            )
    n_params = len(in_names)
    n_outs = len(out_names)
    all_names = in_names + out_names + ([partition_name] if partition_name else [])

    def _body(*args):
        operands = list(args)
        if partition_name is not None:
            operands.append(bass2jax.partition_id_tensor())
        outs = bass2jax._bass_exec_p.bind(
            *operands,
            out_avals=tuple(out_avals),
            in_names=tuple(all_names),
            out_names=tuple(out_names),
            lowering_input_output_aliases=(),
            sim_require_finite=True,
            sim_require_nnan=True,
            nc=nc,
        )
        return tuple(outs)

    devices = jax.devices()[:NCORES]
    mesh = Mesh(np.asarray(devices), ("core",))
    donate = tuple(range(n_params, n_params + n_outs))
    sharded = jax.jit(
        shard_map(
            _body,
            mesh=mesh,
            in_specs=(PartitionSpec("core"),) * (n_params + n_outs),
            out_specs=(PartitionSpec("core"),) * n_outs,
            check_rep=False,
        ),
        donate_argnums=donate,
        keep_unused=True,
    )

    zero_shapes = [tuple(a.shape) for a in out_avals]
    zero_dtypes = [a.dtype for a in out_avals]

    def run(in_maps):
        concat_in = [
            np.concatenate([np.asarray(in_maps[c][n]) for c in range(NCORES)], axis=0)
            for n in in_names
        ]
        concat_zeros = [
            np.zeros((NCORES * s[0], *s[1:]), d)
            for s, d in zip(zero_shapes, zero_dtypes)
        ]
        out_arrs = sharded(*concat_in, *concat_zeros)
        out_arrs = [np.asarray(a) for a in out_arrs]
        return [
            {
                n: out_arrs[i].reshape(NCORES, *zero_shapes[i])[c]
                for i, n in enumerate(out_names)
            }
            for c in range(NCORES)
        ]

    _CACHE["runner"] = run
    return run


def kernel(**inputs):
    run = _get_runner()
    results = run(make_in_maps(inputs))

    out = np.empty((B, P, 512), np.float32)
    bb_tokens = np.empty((B, P, 32), np.float32)
    visual_tokens = np.empty((B, P, 128), np.float32)
    for c in range(NCORES):
        lo = c * BPC
        out[lo : lo + BPC] = results[c]["o_out"].reshape(BPC, P, 512)
        visual_tokens[lo : lo + BPC] = results[c]["o_vt"].reshape(BPC, P, 128)
        bb_tokens[lo : lo + BPC] = results[c]["o_bt"].reshape(BPC, P, 32)
    return out, bb_tokens, visual_tokens
